# revision 14
# baseline (speedup 1.0000x reference)
"""Trainium2 Bass kernel for AttentionSocialPooling.

Strategy (8 cores, data parallel over batch B=8; core m handles batch b=m).
Per (b,t): score s[i,j] = sum_a w2_a*relu(u_a[i]+v_a[j]) + b2 with
u = pos@(W1p-W1d)+b1, v = pos@W1d.  One fp16 PE matmul per t materializes
c_a = e_a*(u_a+v_a) for all (a,i) columns (channel-major: col = a*N+i) via
the delta trick (lhsT rows [1; v], moving rows [u; delta]).

Channels are split into halves A (slots 0:8, value = +contribution) and
B (slots 8:16, value = -contribution); emission sign e_a = +w2 (A), -w2 (B).
ACT evacuates PSUM->SBUF with Relu over contiguous channel blocks (A fully;
the first b_act B-channels); DVE evacuates the rest with tensor_scalar
max / min.  The A-reduction is a bf16 subtract-then-add tree batched over
8 t's: L1 (A-B) and L2 on DVE (2x mode), L3/L4 on GPSIMD; sigmoid per 8 t's
on ACT.

Distance mask softened: the dist matmul (fp16 hi/lo, sqrt(KAPPA) folded per
side) emits z = KAPPA*(R^2-d^2); m = clip(z,0,1) in one DVE tensor_scalar.
The diagonal stays in (m[i,i]=1): its numerator contribution cancels exactly
(pos_j and pos_i use the same fp16-rounded positions) and the count
subtracts 1 in the tail.  Final row sums via PE matmuls with w^T / m^T
stationary, DEFERRED into the next t-group so their w8-dependency never
stalls H-matmuls in the PE FIFO; tail divides by count on DVE.
"""

import numpy as np
import ml_dtypes

B, T, N, C, A = 8, 64, 128, 2, 16
R2 = 2500.0
KAPPA = 16.0
NA = N * A             # 2048 columns per t
HALF = NA // 2
TG = 8                 # t-group size for tree/sigmoid batching
NG = T // TG

bf16 = ml_dtypes.bfloat16
f16 = np.float16

_CACHE = {}


def _plan_channels(w2):
    """Assign 16 channels to halves/engines (channel-major slot layout).

    Slot layout: [A: a_act ACT-pos | a_max DVE-pos | a_min DVE-neg ||
                  B: b_act ACT-neg | b_max DVE-neg | b_min DVE-pos]
    A-half values = +contribution, B-half = -contribution (emission -w2).
    """
    pos = [int(i) for i in np.where(w2 >= 0)[0]]
    neg = [int(i) for i in np.where(w2 < 0)[0]]
    npos, nneg = len(pos), len(neg)
    assert npos + nneg == A

    nBneg = min(nneg, 8)
    nAneg = nneg - nBneg
    nApos = 8 - nAneg

    nBA = min(3, nBneg)

    a_act = pos[:nApos]
    a_dve_max = []
    a_dve_min = neg[nBneg:]
    b_act = neg[:nBA]
    b_dve_max = neg[nBA:nBneg]
    b_dve_min = pos[nApos:]
    order = a_act + a_dve_max + a_dve_min + b_act + b_dve_max + b_dve_min
    assert len(order) == A and sorted(order) == list(range(A))

    emis = np.empty(A, np.float32)
    for s, ch in enumerate(order):
        emis[s] = w2[ch] if s < 8 else -w2[ch]

    plan = dict(
        a_act=len(a_act), a_max=len(a_dve_max), a_min=len(a_dve_min),
        b_act=len(b_act), b_max=len(b_dve_max), b_min=len(b_dve_min),
    )
    return order, emis, plan


def _host_prep(positions, W1, b1, W2, b2):
    pos = np.asarray(positions, dtype=np.float32)
    W1 = np.asarray(W1, dtype=np.float32)
    b1 = np.asarray(b1, dtype=np.float32)
    W2 = np.asarray(W2, dtype=np.float32)
    b2 = np.asarray(b2, dtype=np.float32)

    W1p, W1d = W1[:C], W1[C:]
    w2 = W2[:, 0]
    order, emis, plan = _plan_channels(w2)

    Wu = (W1p - W1d)[:, order] * emis
    Wd = W1d[:, order] * emis
    b1v = b1[order] * emis

    u = (pos @ Wu + b1v).astype(f16)     # [B,T,N,A]
    v = (pos @ Wd).astype(f16)

    vT = np.empty((B, 1 + A, T * N), dtype=f16)
    vT[:, 0] = np.asarray(1.0, dtype=f16)
    vT[:, 1:] = v.transpose(0, 3, 1, 2).reshape(B, A, T * N)

    # channel-major moving row: col = a*N + i  -> u[t, i, a] -> [t, a, i]
    uflat = np.ascontiguousarray(
        u.transpose(0, 1, 3, 2).reshape(B, T, 1, NA))

    # delta pattern [A, N*A] channel-major: delta[a, a*N + i] = 1
    delta = np.zeros((A, NA), dtype=f16)
    for a in range(A):
        delta[a, a * N:(a + 1) * N] = np.asarray(1.0, dtype=f16)

    # soft-mask matmul operands: z = KAPPA*(R2-d2), sqrt(KAPPA) per side
    sk = np.sqrt(KAPPA)
    pos64 = pos.astype(np.float64)
    n2 = (pos64 ** 2).sum(-1)
    px = pos64[..., 0].reshape(B, T * N)
    py = pos64[..., 1].reshape(B, T * N)
    n2f = n2.reshape(B, T * N)

    def hilo(x):
        hi = x.astype(f16)
        lo = (x - hi.astype(np.float64)).astype(f16)
        return hi, lo

    pxh, pxl = hilo(sk * px)
    pyh, pyl = hilo(sk * py)
    n2jh, n2jl = hilo(-sk * n2f)
    p2xh, p2xl = hilo(2 * sk * px)
    p2yh, p2yl = hilo(2 * sk * py)
    n2ih, n2il = hilo(sk * (R2 - n2f))
    skones = np.full_like(pxh, sk)
    lhsTd = np.stack([pxh, pxh, pxl, pyh, pyh, pyl, skones, skones,
                      n2jh, n2jl], axis=1).astype(f16)
    rhsd = np.stack([p2xh, p2xl, p2xh, p2yh, p2yl, p2yh, n2ih, n2il,
                     skones, skones], axis=1).astype(f16)

    pos16 = pos.astype(f16)
    pos3 = np.empty((B, N, T * 3), f16)
    p3 = pos3.reshape(B, N, T, 3)
    p3[..., 0] = pos16[..., 0].transpose(0, 2, 1)
    p3[..., 1] = pos16[..., 1].transpose(0, 2, 1)
    p3[..., 2] = 1.0

    posI = np.empty((B, N, T * 2), np.float32)
    pI = posI.reshape(B, N, T, 2)
    pI[..., 0] = pos16[..., 0].astype(np.float32).transpose(0, 2, 1)
    pI[..., 1] = pos16[..., 1].astype(np.float32).transpose(0, 2, 1)

    return dict(vT=vT, uflat=uflat, delta=delta, lhsTd=lhsTd, rhsd=rhsd,
                pos3=pos3, posI=posI, plan=plan, b2=float(b2[0]))


def _build_program(plan_key, b2val):
    import concourse.bacc as bacc
    import concourse.mybir as mybir
    import concourse.tile as tile

    f32 = mybir.dt.float32
    fp16 = mybir.dt.float16
    bfl = mybir.dt.bfloat16
    Alu = mybir.AluOpType
    Act = mybir.ActivationFunctionType

    (a_act, a_max, a_min, b_act, b_max, b_min) = plan_key
    K2 = 1 + A

    nc = bacc.Bacc()

    vT_p = nc.declare_dram_parameter("vT", [K2, T * N], fp16, isOutput=False)
    uflat_p = nc.declare_dram_parameter("uflat", [T, 1, NA], fp16, isOutput=False)
    delta_p = nc.declare_dram_parameter("delta", [A, NA], fp16, isOutput=False)
    lhsTd_p = nc.declare_dram_parameter("lhsTd", [10, T * N], fp16, isOutput=False)
    rhsd_p = nc.declare_dram_parameter("rhsd", [10, T * N], fp16, isOutput=False)
    pos3_p = nc.declare_dram_parameter("pos3", [N, T * 3], fp16, isOutput=False)
    posI_p = nc.declare_dram_parameter("posI", [N, T * 2], f32, isOutput=False)
    out_p = nc.declare_dram_parameter("out", [T, N, C], f32, isOutput=True)

    with tile.TileContext(nc) as tc:
        with (
            tc.tile_pool(name="pers", bufs=1) as pers,
            tc.tile_pool(name="hpsum", bufs=2, space="PSUM") as hpsum,
            tc.tile_pool(name="dpsum", bufs=2, space="PSUM") as dpsum,
            tc.tile_pool(name="fpsum", bufs=2, space="PSUM") as fpsum,
            tc.tile_pool(name="rwork", bufs=2) as rwork,
            tc.tile_pool(name="swork", bufs=2) as swork,
            tc.tile_pool(name="awork", bufs=2) as awork,
            tc.tile_pool(name="twork", bufs=2) as twork,
        ):
            vT_s = pers.tile([K2, T * N], fp16, tag="vT")
            lhsTd_s = pers.tile([10, T * N], fp16, tag="lhsTd")
            rhsd_s = pers.tile([10, T * N], fp16, tag="rhsd")
            pos3_s = pers.tile([N, T * 3], fp16, tag="pos3")
            posI_s = pers.tile([N, T * 2], f32, tag="posI")
            rhH = [pers.tile([K2, NA], fp16, tag=f"rh{i}", name=f"rh{i}")
                   for i in range(4)]

            nc.gpsimd.dma_start(vT_s[:], vT_p[:])
            nc.gpsimd.dma_start(lhsTd_s[:], lhsTd_p[:])
            nc.gpsimd.dma_start(rhsd_s[:], rhsd_p[:])
            nc.gpsimd.dma_start(pos3_s[:], pos3_p[:])
            nc.gpsimd.dma_start(posI_s[:], posI_p[:])
            for i in range(4):
                nc.gpsimd.dma_start(rhH[i][1:K2, :], delta_p[:])

            pd = None
            R8 = None
            att8 = None
            m8 = None
            w8 = None
            pf = None
            # deferred state from the previous group
            prev = None  # dict(w8=, m8=, pf=, t0=)

            def emit_finals(st):
                for gg in range(TG):
                    tt = st["t0"] + gg
                    s = gg * N
                    nc.tensor.matmul(st["pf"][:, 4 * gg:4 * gg + 3],
                                     st["w8"][:, s:s + N],
                                     pos3_s[:, 3 * tt:3 * tt + 3],
                                     start=True, stop=True)
                    nc.tensor.matmul(st["pf"][:, 4 * gg + 3:4 * gg + 4],
                                     st["m8"][:, s:s + N],
                                     pos3_s[:, 3 * tt + 2:3 * tt + 3],
                                     start=True, stop=True)

            def emit_tail(st):
                t0 = st["t0"]
                pf3 = st["pf"][:].rearrange("p (g c) -> p g c", c=4)
                pI3 = posI_s[:, 2 * t0:2 * (t0 + TG)].rearrange(
                    "p (g c) -> p g c", c=2)
                cnt8 = twork.tile([N, 8], f32, tag="cnt8")
                rcp8 = twork.tile([N, 8], f32, tag="rcp8")
                sw8 = twork.tile([N, 16], f32, tag="sw8")
                outst = twork.tile([N, 16], f32, tag="outst")
                nc.vector.tensor_scalar(cnt8[:], pf3[:, :, 3], -1.0, 1e-6,
                                        op0=Alu.add, op1=Alu.max)
                nc.vector.reciprocal(rcp8[:], cnt8[:])
                s3 = sw8[:].rearrange("p (g c) -> p g c", c=2)
                o3 = outst[:].rearrange("p (g c) -> p g c", c=2)
                for c in range(2):
                    nc.vector.tensor_mul(s3[:, :, c], pf3[:, :, 2],
                                         pI3[:, :, c])
                    nc.vector.tensor_sub(o3[:, :, c], pf3[:, :, c],
                                         s3[:, :, c])
                    nc.vector.tensor_mul(o3[:, :, c], o3[:, :, c], rcp8[:])
                nc.sync.dma_start(
                    out_p[t0:t0 + TG].rearrange("t n c -> n t c"), outst[:])

            for t in range(T):
                g2 = t % 2
                g8 = t % TG
                rh = rhH[t % 4]
                nc.sync.dma_start(rh[0:1, :], uflat_p[t])

                # H matmul, channel-major halves: A = slots 0:8, B = 8:16
                hpA = hpsum.tile([N, HALF], f32, tag="H", name="hpA")
                hpB = hpsum.tile([N, HALF], f32, tag="H", name="hpB")
                lhs = vT_s[:, t * N:(t + 1) * N]
                nc.tensor.matmul(hpA[:, 0:512], lhs, rh[:, 0:512],
                                 start=True, stop=True)
                nc.tensor.matmul(hpA[:, 512:1024], lhs, rh[:, 512:1024],
                                 start=True, stop=True)
                nc.tensor.matmul(hpB[:, 0:512], lhs, rh[:, 1024:1536],
                                 start=True, stop=True)
                nc.tensor.matmul(hpB[:, 512:1024], lhs, rh[:, 1536:2048],
                                 start=True, stop=True)

                # dist z matmul (128 cols per t; [t-even | t-odd] halves)
                if g2 == 0:
                    pd = dpsum.tile([N, 2 * N], f32, tag="D")
                nc.tensor.matmul(pd[:, g2 * N:(g2 + 1) * N],
                                 lhsTd_s[:, t * N:(t + 1) * N],
                                 rhsd_s[:, t * N:(t + 1) * N],
                                 start=True, stop=True)

                # deferred finals/tail from the previous group
                if prev is not None and g8 == 1:
                    emit_finals(prev)
                if prev is not None and g8 == 3:
                    emit_tail(prev)
                    prev = None

                # evacuate into the 8-t batched R tile (signed bf16)
                # R8 layout: [j, (g, a, i)]
                if g8 == 0:
                    R8 = rwork.tile([N, TG * NA], bfl, tag="R8")
                R5 = R8[:].rearrange("p (g a i) -> p g a i", g=TG, a=A)
                base = g8 * NA

                def rg(s0, s1):
                    return R8[:, base + s0 * N:base + s1 * N]

                if a_act > 0:
                    nc.scalar.activation(rg(0, a_act), hpA[:, 0:a_act * N],
                                         Act.Relu)
                if a_max > 0:
                    o = a_act
                    nc.vector.tensor_scalar(rg(o, o + a_max),
                                            hpA[:, o * N:(o + a_max) * N],
                                            0.0, None, op0=Alu.max)
                if a_min > 0:
                    o = a_act + a_max
                    nc.vector.tensor_scalar(rg(o, o + a_min),
                                            hpA[:, o * N:(o + a_min) * N],
                                            0.0, None, op0=Alu.min)
                if b_act > 0:
                    nc.scalar.activation(rg(8, 8 + b_act),
                                         hpB[:, 0:b_act * N], Act.Relu)
                if b_max > 0:
                    o = b_act
                    nc.vector.tensor_scalar(rg(8 + o, 8 + o + b_max),
                                            hpB[:, o * N:(o + b_max) * N],
                                            0.0, None, op0=Alu.max)
                if b_min > 0:
                    o = b_act + b_max
                    nc.vector.tensor_scalar(rg(8 + o, 8 + o + b_min),
                                            hpB[:, o * N:(o + b_min) * N],
                                            0.0, None, op0=Alu.min)

                # per-2t soft mask
                if g2 == 1:
                    if g8 == 1:
                        att8 = awork.tile([N, TG * N], fp16, tag="att8")
                        m8 = awork.tile([N, TG * N], fp16, tag="m8")
                        w8 = awork.tile([N, TG * N], fp16, tag="w8")
                    nc.vector.tensor_scalar(
                        m8[:, (g8 - 1) * N:(g8 + 1) * N], pd[:], 0.0, 1.0,
                        op0=Alu.max, op1=Alu.min)

                if g8 == TG - 1:
                    with nc.allow_low_precision(reason="bf16 channel sum"):
                        S1 = swork.tile([N, TG * NA // 2], bfl, tag="S1")
                        S14 = S1[:].rearrange("p (g a i) -> p g a i",
                                              g=TG, a=8)
                        nc.vector.tensor_tensor(S14[:], R5[:, :, 0:8, :],
                                                R5[:, :, 8:16, :],
                                                op=Alu.subtract)
                        S2 = swork.tile([N, TG * NA // 4], bfl, tag="S2")
                        S24 = S2[:].rearrange("p (g a i) -> p g a i",
                                              g=TG, a=4)
                        nc.vector.tensor_tensor(S24[:], S14[:, :, 0:4, :],
                                                S14[:, :, 4:8, :], op=Alu.add)
                        S3 = swork.tile([N, TG * NA // 8], bfl, tag="S3")
                        S34 = S3[:].rearrange("p (g a i) -> p g a i",
                                              g=TG, a=2)
                        nc.gpsimd.tensor_tensor(S34[:], S24[:, :, 0:2, :],
                                                S24[:, :, 2:4, :], op=Alu.add)
                        S4 = swork.tile([N, TG * N], bfl, tag="S4")
                        nc.gpsimd.tensor_tensor(
                            S4[:].rearrange("p (g i) -> p g i", g=TG),
                            S34[:, :, 0, :], S34[:, :, 1, :], op=Alu.add)

                    nc.scalar.activation(att8[:], S4[:], Act.Sigmoid,
                                         bias=b2val, scale=1.0)
                    nc.gpsimd.tensor_mul(w8[:], att8[:], m8[:])

                    pf = fpsum.tile([N, 4 * TG], f32, tag="F")
                    prev = dict(w8=w8, m8=m8, pf=pf, t0=t - (TG - 1))

            # flush the last group
            emit_finals(prev)
            emit_tail(prev)

    nc.compile()
    return nc


def kernel(positions, W1, b1, W2, b2, _trace=False, _trace_kwargs=None):
    from concourse.bass_utils import run_bass_kernel_spmd

    prep = _host_prep(positions, W1, b1, W2, b2)
    plan = prep["plan"]
    b2v = prep["b2"]
    plan_key = (plan["a_act"], plan["a_max"], plan["a_min"],
                plan["b_act"], plan["b_max"], plan["b_min"])

    key = (plan_key, b2v)
    if key not in _CACHE:
        _CACHE[key] = _build_program(plan_key, b2v)
    nc = _CACHE[key]

    in_maps = []
    for b in range(B):
        in_maps.append({
            "vT": np.ascontiguousarray(prep["vT"][b]),
            "uflat": np.ascontiguousarray(prep["uflat"][b]),
            "delta": prep["delta"],
            "lhsTd": np.ascontiguousarray(prep["lhsTd"][b]),
            "rhsd": np.ascontiguousarray(prep["rhsd"][b]),
            "pos3": np.ascontiguousarray(prep["pos3"][b]),
            "posI": np.ascontiguousarray(prep["posI"][b]),
        })

    kw = {}
    if _trace:
        kw["trace"] = True
        if _trace_kwargs:
            kw.update(_trace_kwargs)
    res = run_bass_kernel_spmd(nc, in_maps, list(range(B)), **kw)
    out = np.stack([r["out"] for r in res.results], axis=0).astype(np.float32)
    if _trace:
        return out, res
    return out


# revision 16
# speedup vs baseline: 1.1987x; 1.1987x over previous
"""Trainium2 Bass kernel for AttentionSocialPooling.

Strategy (8 cores, data parallel over batch B=8; core m handles batch b=m).
Per (b,t): score s[i,j] = sum_a w2_a*relu(u_a[i]+v_a[j]) + b2 with
u = pos@(W1p-W1d)+b1, v = pos@W1d.  One fp16 PE matmul per t materializes
c_a = e_a*(u_a+v_a) for all (a,i) columns (channel-major: col = a*N+i) via
the delta trick (lhsT rows [1; v], moving rows [u; delta]).

Channels are split into halves A (slots 0:8, value = +contribution) and
B (slots 8:16, value = -contribution); emission sign e_a = +w2 (A), -w2 (B).
ACT evacuates PSUM->SBUF with Relu over contiguous channel blocks (A fully;
the first b_act B-channels); DVE evacuates the rest with tensor_scalar
max / min.  The A-reduction is a bf16 subtract-then-add tree batched over
8 t's: L1 (A-B) and L2 on DVE (2x mode), L3/L4 on GPSIMD; sigmoid per 8 t's
on ACT.

Distance mask softened: the dist matmul (fp16 hi/lo, sqrt(KAPPA) folded per
side) emits z = KAPPA*(R^2-d^2); m = clip(z,0,1) in one DVE tensor_scalar.
The diagonal stays in (m[i,i]=1): its numerator contribution cancels exactly
(pos_j and pos_i use the same fp16-rounded positions) and the count
subtracts 1 in the tail.  Final row sums via PE matmuls with w^T / m^T
stationary, DEFERRED into the next t-group so their w8-dependency never
stalls H-matmuls in the PE FIFO; tail divides by count on DVE.
"""

import numpy as np
import ml_dtypes

B, T, N, C, A = 8, 64, 128, 2, 16
R2 = 2500.0
KAPPA = 16.0
NA = N * A             # 2048 columns per t
HALF = NA // 2
TG = 8                 # t-group size for tree/sigmoid batching
NG = T // TG

bf16 = ml_dtypes.bfloat16
f16 = np.float16

_CACHE = {}


def _plan_channels(w2):
    """Assign 16 channels to halves/engines (channel-major slot layout).

    Slot layout: [A: a_act ACT-pos | a_max DVE-pos | a_min DVE-neg ||
                  B: b_act ACT-neg | b_max DVE-neg | b_min DVE-pos]
    A-half values = +contribution, B-half = -contribution (emission -w2).
    """
    pos = [int(i) for i in np.where(w2 >= 0)[0]]
    neg = [int(i) for i in np.where(w2 < 0)[0]]
    npos, nneg = len(pos), len(neg)
    assert npos + nneg == A

    nBneg = min(nneg, 8)
    nAneg = nneg - nBneg
    nApos = 8 - nAneg

    nBA = min(3, nBneg)

    a_act = pos[:nApos]
    a_dve_max = []
    a_dve_min = neg[nBneg:]
    b_act = neg[:nBA]
    b_dve_max = neg[nBA:nBneg]
    b_dve_min = pos[nApos:]
    order = a_act + a_dve_max + a_dve_min + b_act + b_dve_max + b_dve_min
    assert len(order) == A and sorted(order) == list(range(A))

    emis = np.empty(A, np.float32)
    for s, ch in enumerate(order):
        emis[s] = w2[ch] if s < 8 else -w2[ch]

    plan = dict(
        a_act=len(a_act), a_max=len(a_dve_max), a_min=len(a_dve_min),
        b_act=len(b_act), b_max=len(b_dve_max), b_min=len(b_dve_min),
    )
    return order, emis, plan


def _host_prep(positions, W1, b1, W2, b2):
    pos = np.asarray(positions, dtype=np.float32)
    W1 = np.asarray(W1, dtype=np.float32)
    b1 = np.asarray(b1, dtype=np.float32)
    W2 = np.asarray(W2, dtype=np.float32)
    b2 = np.asarray(b2, dtype=np.float32)

    W1p, W1d = W1[:C], W1[C:]
    w2 = W2[:, 0]
    order, emis, plan = _plan_channels(w2)

    Wu = (W1p - W1d)[:, order] * emis
    Wd = W1d[:, order] * emis
    b1v = b1[order] * emis

    u = (pos @ Wu + b1v).astype(f16)     # [B,T,N,A]
    v = (pos @ Wd).astype(f16)

    vT = np.empty((B, 1 + A, T * N), dtype=f16)
    vT[:, 0] = np.asarray(1.0, dtype=f16)
    vT[:, 1:] = v.transpose(0, 3, 1, 2).reshape(B, A, T * N)

    # channel-major moving row: col = a*N + i  -> u[t, i, a] -> [t, a, i]
    uflat = np.ascontiguousarray(
        u.transpose(0, 1, 3, 2).reshape(B, T, 1, NA))

    # delta pattern [A, N*A] channel-major: delta[a, a*N + i] = 1
    delta = np.zeros((A, NA), dtype=f16)
    for a in range(A):
        delta[a, a * N:(a + 1) * N] = np.asarray(1.0, dtype=f16)

    # soft-mask matmul operands: z = KAPPA*(R2-d2), sqrt(KAPPA) per side
    sk = np.sqrt(KAPPA)
    pos64 = pos.astype(np.float64)
    n2 = (pos64 ** 2).sum(-1)
    px = pos64[..., 0].reshape(B, T * N)
    py = pos64[..., 1].reshape(B, T * N)
    n2f = n2.reshape(B, T * N)

    def hilo(x):
        hi = x.astype(f16)
        lo = (x - hi.astype(np.float64)).astype(f16)
        return hi, lo

    pxh, pxl = hilo(sk * px)
    pyh, pyl = hilo(sk * py)
    n2jh, n2jl = hilo(-sk * n2f)
    p2xh, p2xl = hilo(2 * sk * px)
    p2yh, p2yl = hilo(2 * sk * py)
    n2ih, n2il = hilo(sk * (R2 - n2f))
    skones = np.full_like(pxh, sk)
    lhsTd = np.stack([pxh, pxh, pxl, pyh, pyh, pyl, skones, skones,
                      n2jh, n2jl], axis=1).astype(f16)
    rhsd = np.stack([p2xh, p2xl, p2xh, p2yh, p2yl, p2yh, n2ih, n2il,
                     skones, skones], axis=1).astype(f16)

    pos16 = pos.astype(f16)
    pos3 = np.empty((B, N, T * 3), f16)
    p3 = pos3.reshape(B, N, T, 3)
    p3[..., 0] = pos16[..., 0].transpose(0, 2, 1)
    p3[..., 1] = pos16[..., 1].transpose(0, 2, 1)
    p3[..., 2] = 1.0

    posI = np.empty((B, N, T * 2), np.float32)
    pI = posI.reshape(B, N, T, 2)
    pI[..., 0] = pos16[..., 0].astype(np.float32).transpose(0, 2, 1)
    pI[..., 1] = pos16[..., 1].astype(np.float32).transpose(0, 2, 1)

    return dict(vT=vT, uflat=uflat, delta=delta, lhsTd=lhsTd, rhsd=rhsd,
                pos3=pos3, posI=posI, plan=plan, b2=float(b2[0]))


def _build_program(plan_key, b2val):
    import concourse.bacc as bacc
    import concourse.mybir as mybir
    import concourse.tile as tile

    f32 = mybir.dt.float32
    fp16 = mybir.dt.float16
    bfl = mybir.dt.bfloat16
    Alu = mybir.AluOpType
    Act = mybir.ActivationFunctionType

    (a_act, a_max, a_min, b_act, b_max, b_min) = plan_key
    K2 = 1 + A

    nc = bacc.Bacc()

    vT_p = nc.declare_dram_parameter("vT", [K2, T * N], fp16, isOutput=False)
    uflat_p = nc.declare_dram_parameter("uflat", [T, 1, NA], fp16, isOutput=False)
    delta_p = nc.declare_dram_parameter("delta", [A, NA], fp16, isOutput=False)
    lhsTd_p = nc.declare_dram_parameter("lhsTd", [10, T * N], fp16, isOutput=False)
    rhsd_p = nc.declare_dram_parameter("rhsd", [10, T * N], fp16, isOutput=False)
    pos3_p = nc.declare_dram_parameter("pos3", [N, T * 3], fp16, isOutput=False)
    posI_p = nc.declare_dram_parameter("posI", [N, T * 2], f32, isOutput=False)
    out_p = nc.declare_dram_parameter("out", [T, N, C], f32, isOutput=True)

    with tile.TileContext(nc) as tc:
        with (
            tc.tile_pool(name="pers", bufs=1) as pers,
            tc.tile_pool(name="hpsum", bufs=3, space="PSUM") as hpsum,
            tc.tile_pool(name="dpsum", bufs=1, space="PSUM") as dpsum,
            tc.tile_pool(name="fpsum", bufs=1, space="PSUM") as fpsum,
            tc.tile_pool(name="rwork", bufs=2) as rwork,
            tc.tile_pool(name="swork", bufs=2) as swork,
            tc.tile_pool(name="awork", bufs=2) as awork,
            tc.tile_pool(name="twork", bufs=2) as twork,
        ):
            vT_s = pers.tile([K2, T * N], fp16, tag="vT")
            lhsTd_s = pers.tile([10, T * N], fp16, tag="lhsTd")
            rhsd_s = pers.tile([10, T * N], fp16, tag="rhsd")
            pos3_s = pers.tile([N, T * 3], fp16, tag="pos3")
            posI_s = pers.tile([N, T * 2], f32, tag="posI")
            rhH = [pers.tile([K2, NA], fp16, tag=f"rh{i}", name=f"rh{i}")
                   for i in range(4)]

            nc.gpsimd.dma_start(vT_s[:], vT_p[:])
            nc.gpsimd.dma_start(lhsTd_s[:], lhsTd_p[:])
            nc.gpsimd.dma_start(rhsd_s[:], rhsd_p[:])
            nc.gpsimd.dma_start(pos3_s[:], pos3_p[:])
            nc.gpsimd.dma_start(posI_s[:], posI_p[:])
            for i in range(4):
                nc.gpsimd.dma_start(rhH[i][1:K2, :], delta_p[:])

            pd = None
            R8 = None
            att8 = None
            m8 = None
            w8 = None
            pf = None
            # deferred state from the previous group
            prev = None  # dict(w8=, m8=, pf=, t0=)

            def emit_finals(st):
                for gg in range(TG):
                    tt = st["t0"] + gg
                    s = gg * N
                    nc.tensor.matmul(st["pf"][:, 4 * gg:4 * gg + 3],
                                     st["w8"][:, s:s + N],
                                     pos3_s[:, 3 * tt:3 * tt + 3],
                                     start=True, stop=True)
                    nc.tensor.matmul(st["pf"][:, 4 * gg + 3:4 * gg + 4],
                                     st["m8"][:, s:s + N],
                                     pos3_s[:, 3 * tt + 2:3 * tt + 3],
                                     start=True, stop=True)

            def emit_tail(st):
                t0 = st["t0"]
                pf3 = st["pf"][:].rearrange("p (g c) -> p g c", c=4)
                pI3 = posI_s[:, 2 * t0:2 * (t0 + TG)].rearrange(
                    "p (g c) -> p g c", c=2)
                cnt8 = twork.tile([N, 8], f32, tag="cnt8")
                rcp8 = twork.tile([N, 8], f32, tag="rcp8")
                sw8 = twork.tile([N, 16], f32, tag="sw8")
                outst = twork.tile([N, 16], f32, tag="outst")
                nc.vector.tensor_scalar(cnt8[:], pf3[:, :, 3], -1.0, 1e-6,
                                        op0=Alu.add, op1=Alu.max)
                nc.vector.reciprocal(rcp8[:], cnt8[:])
                s3 = sw8[:].rearrange("p (g c) -> p g c", c=2)
                o3 = outst[:].rearrange("p (g c) -> p g c", c=2)
                for c in range(2):
                    nc.vector.tensor_mul(s3[:, :, c], pf3[:, :, 2],
                                         pI3[:, :, c])
                    nc.vector.tensor_sub(o3[:, :, c], pf3[:, :, c],
                                         s3[:, :, c])
                    nc.vector.tensor_mul(o3[:, :, c], o3[:, :, c], rcp8[:])
                nc.sync.dma_start(
                    out_p[t0:t0 + TG].rearrange("t n c -> n t c"), outst[:])

            for t in range(T):
                g2 = t % 2
                g8 = t % TG
                rh = rhH[t % 4]
                nc.sync.dma_start(rh[0:1, :], uflat_p[t])

                # H matmul, channel-major halves: A = slots 0:8, B = 8:16
                hpA = hpsum.tile([N, HALF], f32, tag="H", name="hpA")
                hpB = hpsum.tile([N, HALF], f32, tag="H", name="hpB")
                lhs = vT_s[:, t * N:(t + 1) * N]
                nc.tensor.matmul(hpA[:, 0:512], lhs, rh[:, 0:512],
                                 start=True, stop=True)
                nc.tensor.matmul(hpA[:, 512:1024], lhs, rh[:, 512:1024],
                                 start=True, stop=True)
                nc.tensor.matmul(hpB[:, 0:512], lhs, rh[:, 1024:1536],
                                 start=True, stop=True)
                nc.tensor.matmul(hpB[:, 512:1024], lhs, rh[:, 1536:2048],
                                 start=True, stop=True)

                # dist z matmul (128 cols per t; [t-even | t-odd] halves)
                if g2 == 0:
                    pd = dpsum.tile([N, 2 * N], f32, tag="D")
                nc.tensor.matmul(pd[:, g2 * N:(g2 + 1) * N],
                                 lhsTd_s[:, t * N:(t + 1) * N],
                                 rhsd_s[:, t * N:(t + 1) * N],
                                 start=True, stop=True)

                # deferred finals/tail from the previous group
                if prev is not None and g8 == 3:
                    emit_finals(prev)
                if prev is not None and g8 == 5:
                    emit_tail(prev)
                    prev = None

                # evacuate into the 8-t batched R tile (signed bf16)
                # R8 layout: [j, (g, a, i)]
                if g8 == 0:
                    R8 = rwork.tile([N, TG * NA], bfl, tag="R8")
                R5 = R8[:].rearrange("p (g a i) -> p g a i", g=TG, a=A)
                base = g8 * NA

                def rg(s0, s1):
                    return R8[:, base + s0 * N:base + s1 * N]

                if a_act > 0:
                    nc.scalar.activation(rg(0, a_act), hpA[:, 0:a_act * N],
                                         Act.Relu)
                if a_max > 0:
                    o = a_act
                    nc.vector.tensor_scalar(rg(o, o + a_max),
                                            hpA[:, o * N:(o + a_max) * N],
                                            0.0, None, op0=Alu.max)
                if a_min > 0:
                    o = a_act + a_max
                    nc.vector.tensor_scalar(rg(o, o + a_min),
                                            hpA[:, o * N:(o + a_min) * N],
                                            0.0, None, op0=Alu.min)
                if b_act > 0:
                    nc.scalar.activation(rg(8, 8 + b_act),
                                         hpB[:, 0:b_act * N], Act.Relu)
                if b_max > 0:
                    o = b_act
                    nc.vector.tensor_scalar(rg(8 + o, 8 + o + b_max),
                                            hpB[:, o * N:(o + b_max) * N],
                                            0.0, None, op0=Alu.max)
                if b_min > 0:
                    o = b_act + b_max
                    nc.vector.tensor_scalar(rg(8 + o, 8 + o + b_min),
                                            hpB[:, o * N:(o + b_min) * N],
                                            0.0, None, op0=Alu.min)

                # per-2t soft mask
                if g2 == 1:
                    if g8 == 1:
                        att8 = awork.tile([N, TG * N], fp16, tag="att8")
                        m8 = awork.tile([N, TG * N], fp16, tag="m8")
                        w8 = awork.tile([N, TG * N], fp16, tag="w8")
                    nc.vector.tensor_scalar(
                        m8[:, (g8 - 1) * N:(g8 + 1) * N], pd[:], 0.0, 1.0,
                        op0=Alu.max, op1=Alu.min)

                if g8 == TG - 1:
                    with nc.allow_low_precision(reason="bf16 channel sum"):
                        S1 = swork.tile([N, TG * NA // 2], bfl, tag="S1")
                        S14 = S1[:].rearrange("p (g a i) -> p g a i",
                                              g=TG, a=8)
                        nc.vector.tensor_tensor(S14[:], R5[:, :, 0:8, :],
                                                R5[:, :, 8:16, :],
                                                op=Alu.subtract)
                        S2 = swork.tile([N, TG * NA // 4], bfl, tag="S2")
                        S24 = S2[:].rearrange("p (g a i) -> p g a i",
                                              g=TG, a=4)
                        nc.vector.tensor_tensor(S24[:], S14[:, :, 0:4, :],
                                                S14[:, :, 4:8, :], op=Alu.add)
                        S3 = swork.tile([N, TG * NA // 8], bfl, tag="S3")
                        S34 = S3[:].rearrange("p (g a i) -> p g a i",
                                              g=TG, a=2)
                        nc.gpsimd.tensor_tensor(S34[:], S24[:, :, 0:2, :],
                                                S24[:, :, 2:4, :], op=Alu.add)
                        S4 = swork.tile([N, TG * N], bfl, tag="S4")
                        nc.gpsimd.tensor_tensor(
                            S4[:].rearrange("p (g i) -> p g i", g=TG),
                            S34[:, :, 0, :], S34[:, :, 1, :], op=Alu.add)

                    nc.scalar.activation(att8[:], S4[:], Act.Sigmoid,
                                         bias=b2val, scale=1.0)
                    nc.gpsimd.tensor_mul(w8[:], att8[:], m8[:])

                    pf = fpsum.tile([N, 4 * TG], f32, tag="F")
                    prev = dict(w8=w8, m8=m8, pf=pf, t0=t - (TG - 1))

            # flush the last group
            emit_finals(prev)
            emit_tail(prev)

    nc.compile()
    return nc


def kernel(positions, W1, b1, W2, b2, _trace=False, _trace_kwargs=None):
    from concourse.bass_utils import run_bass_kernel_spmd

    prep = _host_prep(positions, W1, b1, W2, b2)
    plan = prep["plan"]
    b2v = prep["b2"]
    plan_key = (plan["a_act"], plan["a_max"], plan["a_min"],
                plan["b_act"], plan["b_max"], plan["b_min"])

    key = (plan_key, b2v)
    if key not in _CACHE:
        _CACHE[key] = _build_program(plan_key, b2v)
    nc = _CACHE[key]

    in_maps = []
    for b in range(B):
        in_maps.append({
            "vT": np.ascontiguousarray(prep["vT"][b]),
            "uflat": np.ascontiguousarray(prep["uflat"][b]),
            "delta": prep["delta"],
            "lhsTd": np.ascontiguousarray(prep["lhsTd"][b]),
            "rhsd": np.ascontiguousarray(prep["rhsd"][b]),
            "pos3": np.ascontiguousarray(prep["pos3"][b]),
            "posI": np.ascontiguousarray(prep["posI"][b]),
        })

    kw = {}
    if _trace:
        kw["trace"] = True
        if _trace_kwargs:
            kw.update(_trace_kwargs)
    res = run_bass_kernel_spmd(nc, in_maps, list(range(B)), **kw)
    out = np.stack([r["out"] for r in res.results], axis=0).astype(np.float32)
    if _trace:
        return out, res
    return out
